# revision 31
# baseline (speedup 1.0000x reference)
"""GPTQ 4-bit quantized linear: out = x @ dequant(qweight, qzeros, scales, g_idx) + bias.

Full shapes: x [8192, 4096] fp16, qweight [512, 4096] int32 (8x 4-bit packed
along K), qzeros [32, 512] int32, scales [32, 4096] fp16, g_idx [4096] int32
(k // 128), bias [4096] fp16.  Output [8192, 4096] fp16.

Strategy: 2 (M) x 4 (N) grid over 8 NeuronCores.  Per core: M=4096, N=1024,
K=4096, all in fp16 on the PE -- but via one level of Strassen, which cuts
PE cycles to 7/8 of the direct matmul (the PE at 78.6 TF/s fp16 is the
bottleneck; fp8 DoubleRow was measured to give 2x FLOPs per cycle but the
3-term error-corrected decomposition it needs costs 3x FLOPs, a net loss).

Per core split M=2x2048, K=2x2048, N=2x512:
  P1=(A11+A22)(B11+B22)  P2=(A21+A22)B11  P3=A11(B12-B22)  P4=A22(B21-B11)
  P5=(A11+A12)B22        P6=(A21-A11)(B11+B12)  P7=(A12-A22)(B21+B22)
  C11=P1+P4-P5+P7  C12=P3+P5  C21=P2+P4  C22=P1-P2+P3+P6

The host dequantizes W and precomputes the 7 fp16 A- and B-combos (adds of
quadrants; psum stays fp32 so the fp16 combo rounding is benign -- measured
rel err ~1.2e-3).  The device sweeps the products GLOBALLY product-major:
for each product p, one 16KB/partition B[p] tile is resident while its A
tiles stream across all 16 m'-blocks (16 matmuls of [128k,128m]x[128k,512n]
per block).  This makes startup wait only for B[0]+A[0,0] (~2.6MB) instead
of all 25MB of combos, and leaves the steady state with zero PE stalls.
Per-block drains keep 4 running fp16 accumulators (one per C quadrant)
updated by single-PSUM-operand tensor_tensor ops whose subtraction order
needs no negation pass; bias rides the init op:
  C11 = P7-(P5-(P1+b0+P4))   C12 = (P3+b1)+P5
  C21 = (P2+b0)+P4           C22 = P6+(P3-(P2-(P1+b1)))
"""

import os
import sys

import numpy as np

for _p in ("/opt/trn_rl_repo",):
    if _p not in sys.path and os.path.isdir(_p):
        sys.path.insert(0, _p)

import concourse.bass as bass
import concourse.mybir as mybir
import concourse.tile as tile
from concourse import bacc
from concourse.bass_utils import run_bass_kernel_spmd

dt = mybir.dt

P = 128          # partitions
JP = 8           # 4-bit values per int32
GROUP = 128      # quant group size
NPS = 512        # psum free width / n'-quadrant width
NPROD = 7


def build_program(K, M, N):
    """One-core SPMD program: Strassen 1-level over [M=4096,K=4096,N=1024]."""
    KH, MH, NH = K // 2, M // 2, N // 2
    KT = KH // P         # 16 k'-tiles per product
    MB = MH // P         # 16 m'-blocks
    assert NH == NPS

    nc = bacc.Bacc("TRN2", target_bir_lowering=False)

    ac = nc.dram_tensor("ac", [NPROD, MB, P, KT, P], dt.float16, kind="ExternalInput")
    bc = nc.dram_tensor("bc", [NPROD, P, KT, NPS], dt.float16, kind="ExternalInput")
    bs = nc.dram_tensor("bs", [P, N], dt.float16, kind="ExternalInput")
    out = nc.dram_tensor("out", [M, N], dt.float16, kind="ExternalOutput")

    add = mybir.AluOpType.add
    sub = mybir.AluOpType.subtract

    from contextlib import ExitStack

    with tile.TileContext(nc) as tc, ExitStack() as ctx:
        const = ctx.enter_context(tc.tile_pool(name="const", bufs=1))
        bpool = ctx.enter_context(tc.tile_pool(name="bpool", bufs=2))
        apool = ctx.enter_context(tc.tile_pool(name="apool", bufs=12))
        cpool = ctx.enter_context(tc.tile_pool(name="cpool", bufs=18))
        opool = ctx.enter_context(tc.tile_pool(name="opool", bufs=6))
        psum = ctx.enter_context(tc.tile_pool(name="psum", bufs=8, space="PSUM"))

        # PE warmup: dummy fp16 matmuls with no DMA dependency so the HAM
        # clock-gate opens (1.2 -> 2.4 GHz) before the first real matmul.
        warm_src = const.tile([P, NPS], dt.float16)
        nc.gpsimd.memset(warm_src[:], 0.0)
        warm_ps = psum.tile([P, NPS], dt.float32, tag="ps")
        NWARM = 26   # sized so warmup hands off to the first real matmul
        for wi in range(NWARM):   # (~18.5us) with no idle gap re-gating the clock
            nc.tensor.matmul(
                warm_ps[:], warm_src[:, :P], warm_src[:],
                start=(wi == 0), stop=(wi == NWARM - 1),
            )

        bias_t = const.tile([P, N], dt.float16)
        nc.sync.dma_start(bias_t[:], bs[:])
        b0 = bias_t[:, 0:NPS]
        b1 = bias_t[:, NPS:N]

        # Global product-major sweep: for each product p, stream its A tiles
        # across all 16 m'-blocks against the single resident B[p] tile
        # (16KB/partition, double-buffered).  Startup only waits for
        # B[0]+A[0,0] (~2.6MB); B prefetch needs just 38GB/s.  Per-block
        # drains maintain 4 running fp16 accumulators via tensor_tensor ops
        # with ONE psum operand each; subtractions are ordered so the sign
        # works out with no negation pass, and bias rides the init op:
        #   C11 = P7-(P5-(P1+b0+P4));  C12 = (P3+b1)+P5;  C21 = (P2+b0)+P4
        #   C22 = P6+(P3-(P2-(P1+b1)))
        a11 = [None] * MB; a12 = [None] * MB
        a21 = [None] * MB; a22 = [None] * MB

        def acc(tag, mb, in0, in1, op):
            t = cpool.tile([P, NPS], dt.float16, tag=tag,
                           name=f"{tag}_{mb}", bufs=18)
            nc.vector.tensor_tensor(t[:], in0, in1, op=op)
            return t

        def store(mb, u, nsl, upper):
            mrow = (mb * P) if upper else (MH + mb * P)
            nc.scalar.dma_start(out[mrow:mrow + P, nsl], u[:])

        for prod in range(NPROD):
            bt = bpool.tile([P, KT, NPS], dt.float16, tag="b", name=f"b{prod}")
            # quarter-DMAs alternating rings: subtile deps let the kt=0
            # matmuls start as soon as the first quarter lands (~3us).  For
            # the first product, A(0,0) is issued on sync BEFORE any B bytes
            # (it gates the very first matmul) and the odd B quarters ride
            # sync behind it; later A(0,odd) stay ahead of B on gpsimd.
            KTQ = KT // 4
            at0 = None
            if prod == 0:
                at0 = apool.tile([P, KT, P], dt.float16, tag="a",
                                 name="a_t0_0", bufs=12)
                nc.sync.dma_start(at0[:, 0:KT // 2, :], ac[0, 0, :, 0:KT // 2, :])
                nc.scalar.dma_start(at0[:, KT // 2:, :], ac[0, 0, :, KT // 2:, :])
                engs = [nc.gpsimd, nc.scalar, nc.scalar, nc.sync]
            else:
                engs = [nc.gpsimd, nc.scalar, nc.gpsimd, nc.scalar]
            nq0 = 3 if prod == 0 else 4
            for q in range(nq0):
                engs[q].dma_start(bt[:, q * KTQ:(q + 1) * KTQ, :],
                                  bc[prod, :, q * KTQ:(q + 1) * KTQ, :])
            def a_tile(mb):
                if at0 is not None and mb == 0:
                    return at0
                t = apool.tile([P, KT, P], dt.float16, tag="a",
                               name=f"a_t{prod}_{mb}", bufs=12)
                eng = nc.sync if mb % 2 == 0 else nc.gpsimd
                eng.dma_start(t[:], ac[prod, mb])
                return t

            # Product 0 is ring-ramp-bound: its first blocks would stall
            # ~6us inside block 0 waiting for B[0]'s later quarters.  Run
            # kt 0..7 of blocks 0..3 first (psum groups left open across
            # four banks), then close each with kt 8..15 as the later B
            # quarters land -- the stall becomes useful work.
            NSPL = 4 if prod == 0 else 0
            prologue = []
            for mb in range(NSPL):
                at = a_tile(mb)
                ps = psum.tile([P, NPS], dt.float32, tag="ps",
                               name=f"ps{prod}_{mb}")
                prologue.append((at, ps))
            if prod == 0:
                # deferred: q3 is only read by the kt12-15 matmuls emitted
                # in the main loop below, so it may ride sync AFTER the
                # pass-1 A tiles it would otherwise delay
                engs[3].dma_start(bt[:, 3 * KTQ:4 * KTQ, :],
                                  bc[prod, :, 3 * KTQ:4 * KTQ, :])
            for ktq in range(2):
                for mb in range(NSPL):
                    at, ps = prologue[mb]
                    for kt in range(ktq * KTQ, (ktq + 1) * KTQ):
                        nc.tensor.matmul(
                            ps[:], at[:, kt, :], bt[:, kt, :],
                            start=(kt == 0), stop=False,
                        )

            for mb in range(MB):
                if mb < NSPL:
                    at, ps = prologue[mb]
                    kts = range(KT // 2, KT)
                    first = False
                else:
                    at = a_tile(mb)
                    ps = psum.tile([P, NPS], dt.float32, tag="ps",
                                   name=f"ps{prod}_{mb}")
                    kts = range(KT)
                    first = True
                for kt in kts:
                    nc.tensor.matmul(
                        ps[:], at[:, kt, :], bt[:, kt, :],
                        start=(first and kt == 0), stop=(kt == KT - 1),
                    )
                add = mybir.AluOpType.add
                sub = mybir.AluOpType.subtract
                if prod == 0:      # P1
                    a11[mb] = acc("c11", mb, ps[:], b0, add)
                    a22[mb] = acc("c22", mb, ps[:], b1, add)
                elif prod == 1:    # P2
                    a21[mb] = acc("c21", mb, ps[:], b0, add)
                    a22[mb] = acc("c22", mb, ps[:], a22[mb][:], sub)
                elif prod == 2:    # P3
                    a12[mb] = acc("c12", mb, ps[:], b1, add)
                    a22[mb] = acc("c22", mb, ps[:], a22[mb][:], sub)
                elif prod == 3:    # P4
                    a11[mb] = acc("c11", mb, ps[:], a11[mb][:], add)
                    o21 = opool.tile([P, NPS], dt.float16, tag="o",
                                     name="o21")
                    nc.vector.tensor_tensor(o21[:], ps[:], a21[mb][:], op=add)
                    store(mb, o21, slice(0, NPS), False)
                elif prod == 4:    # P5
                    a11[mb] = acc("c11", mb, ps[:], a11[mb][:], sub)
                    o12 = opool.tile([P, NPS], dt.float16, tag="o",
                                     name="o12")
                    nc.vector.tensor_tensor(o12[:], ps[:], a12[mb][:], op=add)
                    store(mb, o12, slice(NPS, N), True)
                elif prod == 5:    # P6
                    o22 = opool.tile([P, NPS], dt.float16, tag="o",
                                     name="o22")
                    nc.vector.tensor_tensor(o22[:], ps[:], a22[mb][:], op=add)
                    store(mb, o22, slice(NPS, N), False)
                else:              # P7
                    o11 = opool.tile([P, NPS], dt.float16, tag="o",
                                     name="o11")
                    nc.vector.tensor_tensor(o11[:], ps[:], a11[mb][:], op=sub)
                    store(mb, o11, slice(0, NPS), True)
    nc.finalize()
    return nc


def _pack_a(combo):
    """combo: [MH, KH] fp16 -> [MB, 128, KT, 128] with m=128*mb+mm, k=128*kt+p."""
    MH, KH = combo.shape
    a = combo.reshape(MH // P, P, KH // P, P)            # (mb, mm, kt, p)
    return np.ascontiguousarray(a.transpose(0, 3, 2, 1))  # (mb, p, kt, mm)


def _pack_b(combo):
    """combo: [KH, NPS] fp16 -> [128, KT, NPS] with k=128*kt+p."""
    KH = combo.shape[0]
    a = combo.reshape(KH // P, P, NPS)                   # (kt, p, n)
    return np.ascontiguousarray(a.transpose(1, 0, 2))    # (p, kt, n)


def host_prep(x, qweight, qzeros, scales, g_idx, bias, m_split, n_split):
    """Dequantize W, build fp16 Strassen A/B combos, pack per core."""
    M_full, K = x.shape
    G, N_full = scales.shape
    Mc = M_full // m_split
    Nc = N_full // n_split
    KH, MHc, NHc = K // 2, Mc // 2, Nc // 2

    shifts = (np.arange(JP, dtype=np.int32) * 4)
    w = ((qweight[:, None, :] >> shifts[None, :, None]) & 15).reshape(K, N_full)
    z = ((qzeros[:, :, None] >> shifts[None, None, :]) & 15).reshape(G, N_full) + 1
    cg = np.asarray(g_idx[::GROUP])
    assert np.array_equal(np.repeat(cg, GROUP), np.asarray(g_idx)), \
        "g_idx must be uniform within 128-wide k chunks"
    iw = (w - z[cg].repeat(GROUP, axis=0)).astype(np.float32)
    W16 = (iw * np.asarray(scales, np.float32)[cg].repeat(GROUP, axis=0)
           ).astype(np.float16)

    x = np.asarray(x)
    bias = np.asarray(bias)

    a_shards = []
    for mi in range(m_split):
        xm = x[mi * Mc:(mi + 1) * Mc]
        A11 = xm[:MHc, :KH]; A12 = xm[:MHc, KH:]
        A21 = xm[MHc:, :KH]; A22 = xm[MHc:, KH:]
        combos = (A11 + A22, A21 + A22, A11, A22,
                  A11 + A12, A21 - A11, A12 - A22)
        arr = np.empty((NPROD, MHc // P, P, KH // P, P), np.float16)
        for i, c in enumerate(combos):
            arr[i] = _pack_a(np.ascontiguousarray(c))
        a_shards.append(arr)

    b_shards = []
    for ni in range(n_split):
        Wc = W16[:, ni * Nc:(ni + 1) * Nc]
        B11 = Wc[:KH, :NHc]; B12 = Wc[:KH, NHc:]
        B21 = Wc[KH:, :NHc]; B22 = Wc[KH:, NHc:]
        combos = (B11 + B22, B11, B12 - B22, B21 - B11,
                  B22, B11 + B12, B21 + B22)
        arr = np.empty((NPROD, P, KH // P, NHc), np.float16)
        for i, c in enumerate(combos):
            arr[i] = _pack_b(np.ascontiguousarray(c))
        b_shards.append(arr)

    in_maps = []
    for mi in range(m_split):
        for ni in range(n_split):
            in_maps.append({
                "ac": a_shards[mi],
                "bc": b_shards[ni],
                "bs": np.ascontiguousarray(
                    np.broadcast_to(bias[ni * Nc:(ni + 1) * Nc], (P, Nc))
                ),
            })
    return in_maps, Mc, Nc


_PROGRAM_CACHE = {}


def _get_program(K, M, N):
    key = (K, M, N)
    if key not in _PROGRAM_CACHE:
        _PROGRAM_CACHE[key] = build_program(K, M, N)
    return _PROGRAM_CACHE[key]


def kernel(x, qweight, qzeros, scales, g_idx, bias, trace=False, trace_kwargs=None):
    m_split, n_split = 2, 4
    x = np.asarray(x)
    qweight = np.asarray(qweight)
    qzeros = np.asarray(qzeros)
    scales = np.asarray(scales)
    g_idx = np.asarray(g_idx)
    bias = np.asarray(bias)
    M_full, K = x.shape
    N_full = scales.shape[1]
    in_maps, M, N = host_prep(x, qweight, qzeros, scales, g_idx, bias,
                              m_split, n_split)
    nc = _get_program(K, M, N)
    kw = {}
    if trace:
        kw = dict(trace=True, **(trace_kwargs or {}))
    rb = run_bass_kernel_spmd(nc, in_maps, list(range(m_split * n_split)), **kw)
    out = np.empty((M_full, N_full), dtype=np.float16)
    ci = 0
    for mi in range(m_split):
        for ni in range(n_split):
            out[mi * M:(mi + 1) * M, ni * N:(ni + 1) * N] = rb.results[ci]["out"]
            ci += 1
    kernel.last_results = rb
    return out


# revision 32
# speedup vs baseline: 1.0059x; 1.0059x over previous
"""GPTQ 4-bit quantized linear: out = x @ dequant(qweight, qzeros, scales, g_idx) + bias.

Full shapes: x [8192, 4096] fp16, qweight [512, 4096] int32 (8x 4-bit packed
along K), qzeros [32, 512] int32, scales [32, 4096] fp16, g_idx [4096] int32
(k // 128), bias [4096] fp16.  Output [8192, 4096] fp16.

Strategy: 2 (M) x 4 (N) grid over 8 NeuronCores.  Per core: M=4096, N=1024,
K=4096, all in fp16 on the PE -- but via one level of Strassen, which cuts
PE cycles to 7/8 of the direct matmul (the PE at 78.6 TF/s fp16 is the
bottleneck; fp8 DoubleRow was measured to give 2x FLOPs per cycle but the
3-term error-corrected decomposition it needs costs 3x FLOPs, a net loss).

Per core split M=2x2048, K=2x2048, N=2x512:
  P1=(A11+A22)(B11+B22)  P2=(A21+A22)B11  P3=A11(B12-B22)  P4=A22(B21-B11)
  P5=(A11+A12)B22        P6=(A21-A11)(B11+B12)  P7=(A12-A22)(B21+B22)
  C11=P1+P4-P5+P7  C12=P3+P5  C21=P2+P4  C22=P1-P2+P3+P6

The host dequantizes W and precomputes the 7 fp16 A- and B-combos (adds of
quadrants; psum stays fp32 so the fp16 combo rounding is benign -- measured
rel err ~1.2e-3).  The device sweeps the products GLOBALLY product-major:
for each product p, one 16KB/partition B[p] tile is resident while its A
tiles stream across all 16 m'-blocks (16 matmuls of [128k,128m]x[128k,512n]
per block).  This makes startup wait only for B[0]+A[0,0] (~2.6MB) instead
of all 25MB of combos, and leaves the steady state with zero PE stalls.
Per-block drains keep 4 running fp16 accumulators (one per C quadrant)
updated by single-PSUM-operand tensor_tensor ops whose subtraction order
needs no negation pass; bias rides the init op:
  C11 = P7-(P5-(P1+b0+P4))   C12 = (P3+b1)+P5
  C21 = (P2+b0)+P4           C22 = P6+(P3-(P2-(P1+b1)))
"""

import os
import sys

import numpy as np

for _p in ("/opt/trn_rl_repo",):
    if _p not in sys.path and os.path.isdir(_p):
        sys.path.insert(0, _p)

import concourse.bass as bass
import concourse.mybir as mybir
import concourse.tile as tile
from concourse import bacc
from concourse.bass_utils import run_bass_kernel_spmd

dt = mybir.dt

P = 128          # partitions
JP = 8           # 4-bit values per int32
GROUP = 128      # quant group size
NPS = 512        # psum free width / n'-quadrant width
NPROD = 7


def build_program(K, M, N):
    """One-core SPMD program: Strassen 1-level over [M=4096,K=4096,N=1024]."""
    KH, MH, NH = K // 2, M // 2, N // 2
    KT = KH // P         # 16 k'-tiles per product
    MB = MH // P         # 16 m'-blocks
    assert NH == NPS

    nc = bacc.Bacc("TRN2", target_bir_lowering=False)

    ac = nc.dram_tensor("ac", [NPROD, MB, P, KT, P], dt.float16, kind="ExternalInput")
    bc = nc.dram_tensor("bc", [NPROD, P, KT, NPS], dt.float16, kind="ExternalInput")
    bs = nc.dram_tensor("bs", [P, N], dt.float16, kind="ExternalInput")
    out = nc.dram_tensor("out", [M, N], dt.float16, kind="ExternalOutput")

    add = mybir.AluOpType.add
    sub = mybir.AluOpType.subtract

    from contextlib import ExitStack

    with tile.TileContext(nc) as tc, ExitStack() as ctx:
        const = ctx.enter_context(tc.tile_pool(name="const", bufs=1))
        bpool = ctx.enter_context(tc.tile_pool(name="bpool", bufs=2))
        apool = ctx.enter_context(tc.tile_pool(name="apool", bufs=12))
        cpool = ctx.enter_context(tc.tile_pool(name="cpool", bufs=18))
        opool = ctx.enter_context(tc.tile_pool(name="opool", bufs=6))
        psum = ctx.enter_context(tc.tile_pool(name="psum", bufs=8, space="PSUM"))

        # PE warmup: dummy fp16 matmuls with no DMA dependency so the HAM
        # clock-gate opens (1.2 -> 2.4 GHz) before the first real matmul.
        warm_src = const.tile([P, NPS], dt.float16)
        nc.gpsimd.memset(warm_src[:], 0.0)
        warm_ps = psum.tile([P, NPS], dt.float32, tag="ps")
        NWARM = 26   # sized so warmup hands off to the first real matmul
        for wi in range(NWARM):   # (~18.5us) with no idle gap re-gating the clock
            nc.tensor.matmul(
                warm_ps[:], warm_src[:, :P], warm_src[:],
                start=(wi == 0), stop=(wi == NWARM - 1),
            )

        bias_t = const.tile([P, N], dt.float16)
        nc.sync.dma_start(bias_t[:], bs[:])
        b0 = bias_t[:, 0:NPS]
        b1 = bias_t[:, NPS:N]

        # Global product-major sweep: for each product p, stream its A tiles
        # across all 16 m'-blocks against the single resident B[p] tile
        # (16KB/partition, double-buffered).  Startup only waits for
        # B[0]+A[0,0] (~2.6MB); B prefetch needs just 38GB/s.  Per-block
        # drains maintain 4 running fp16 accumulators via tensor_tensor ops
        # with ONE psum operand each; subtractions are ordered so the sign
        # works out with no negation pass, and bias rides the init op:
        #   C11 = P7-(P5-(P1+b0+P4));  C12 = (P3+b1)+P5;  C21 = (P2+b0)+P4
        #   C22 = P6+(P3-(P2-(P1+b1)))
        a11 = [None] * MB; a12 = [None] * MB
        a21 = [None] * MB; a22 = [None] * MB

        def acc(tag, mb, in0, in1, op):
            t = cpool.tile([P, NPS], dt.float16, tag=tag,
                           name=f"{tag}_{mb}", bufs=18)
            nc.vector.tensor_tensor(t[:], in0, in1, op=op)
            return t

        def store(mb, u, nsl, upper):
            mrow = (mb * P) if upper else (MH + mb * P)
            nc.scalar.dma_start(out[mrow:mrow + P, nsl], u[:])

        for prod in range(NPROD):
            bt = bpool.tile([P, KT, NPS], dt.float16, tag="b", name=f"b{prod}")
            # quarter-DMAs alternating rings: subtile deps let the kt=0
            # matmuls start as soon as the first quarter lands (~3us).  For
            # the first product, A(0,0) is issued on sync BEFORE any B bytes
            # (it gates the very first matmul) and the odd B quarters ride
            # sync behind it; later A(0,odd) stay ahead of B on gpsimd.
            KTQ = KT // 4
            at0 = None
            if prod == 0:
                at0 = apool.tile([P, KT, P], dt.float16, tag="a",
                                 name="a_t0_0", bufs=12)
                nc.sync.dma_start(at0[:, 0:KT // 2, :], ac[0, 0, :, 0:KT // 2, :])
                nc.scalar.dma_start(at0[:, KT // 2:, :], ac[0, 0, :, KT // 2:, :])
                engs = [nc.gpsimd, nc.scalar, nc.scalar, nc.sync]
            else:
                engs = [nc.gpsimd, nc.scalar, nc.gpsimd, nc.scalar]
            for q in range(4):
                engs[q].dma_start(bt[:, q * KTQ:(q + 1) * KTQ, :],
                                  bc[prod, :, q * KTQ:(q + 1) * KTQ, :])
            def a_tile(mb):
                if at0 is not None and mb == 0:
                    return at0
                t = apool.tile([P, KT, P], dt.float16, tag="a",
                               name=f"a_t{prod}_{mb}", bufs=12)
                eng = nc.sync if mb % 2 == 0 else nc.gpsimd
                eng.dma_start(t[:], ac[prod, mb])
                return t

            # Product 0 is ring-ramp-bound: its first blocks would stall
            # ~6us inside block 0 waiting for B[0]'s later quarters.  Run
            # kt 0..7 of blocks 0..3 first (psum groups left open across
            # four banks), then close each with kt 8..15 as the later B
            # quarters land -- the stall becomes useful work.
            NSPL = 4 if prod == 0 else 0
            prologue = []
            for mb in range(NSPL):
                at = a_tile(mb)
                ps = psum.tile([P, NPS], dt.float32, tag="ps",
                               name=f"ps{prod}_{mb}")
                for kt in range(KT // 2):
                    nc.tensor.matmul(
                        ps[:], at[:, kt, :], bt[:, kt, :],
                        start=(kt == 0), stop=False,
                    )
                prologue.append((at, ps))

            for mb in range(MB):
                if mb < NSPL:
                    at, ps = prologue[mb]
                    kts = range(KT // 2, KT)
                    first = False
                else:
                    at = a_tile(mb)
                    ps = psum.tile([P, NPS], dt.float32, tag="ps",
                                   name=f"ps{prod}_{mb}")
                    kts = range(KT)
                    first = True
                for kt in kts:
                    nc.tensor.matmul(
                        ps[:], at[:, kt, :], bt[:, kt, :],
                        start=(first and kt == 0), stop=(kt == KT - 1),
                    )
                add = mybir.AluOpType.add
                sub = mybir.AluOpType.subtract
                if prod == 0:      # P1
                    a11[mb] = acc("c11", mb, ps[:], b0, add)
                    a22[mb] = acc("c22", mb, ps[:], b1, add)
                elif prod == 1:    # P2
                    a21[mb] = acc("c21", mb, ps[:], b0, add)
                    a22[mb] = acc("c22", mb, ps[:], a22[mb][:], sub)
                elif prod == 2:    # P3
                    a12[mb] = acc("c12", mb, ps[:], b1, add)
                    a22[mb] = acc("c22", mb, ps[:], a22[mb][:], sub)
                elif prod == 3:    # P4
                    a11[mb] = acc("c11", mb, ps[:], a11[mb][:], add)
                    o21 = opool.tile([P, NPS], dt.float16, tag="o",
                                     name="o21")
                    nc.vector.tensor_tensor(o21[:], ps[:], a21[mb][:], op=add)
                    store(mb, o21, slice(0, NPS), False)
                elif prod == 4:    # P5
                    a11[mb] = acc("c11", mb, ps[:], a11[mb][:], sub)
                    o12 = opool.tile([P, NPS], dt.float16, tag="o",
                                     name="o12")
                    nc.vector.tensor_tensor(o12[:], ps[:], a12[mb][:], op=add)
                    store(mb, o12, slice(NPS, N), True)
                elif prod == 5:    # P6
                    o22 = opool.tile([P, NPS], dt.float16, tag="o",
                                     name="o22")
                    nc.vector.tensor_tensor(o22[:], ps[:], a22[mb][:], op=add)
                    store(mb, o22, slice(NPS, N), False)
                else:              # P7
                    o11 = opool.tile([P, NPS], dt.float16, tag="o",
                                     name="o11")
                    nc.vector.tensor_tensor(o11[:], ps[:], a11[mb][:], op=sub)
                    store(mb, o11, slice(0, NPS), True)
    nc.finalize()
    return nc


def _pack_a(combo):
    """combo: [MH, KH] fp16 -> [MB, 128, KT, 128] with m=128*mb+mm, k=128*kt+p."""
    MH, KH = combo.shape
    a = combo.reshape(MH // P, P, KH // P, P)            # (mb, mm, kt, p)
    return np.ascontiguousarray(a.transpose(0, 3, 2, 1))  # (mb, p, kt, mm)


def _pack_b(combo):
    """combo: [KH, NPS] fp16 -> [128, KT, NPS] with k=128*kt+p."""
    KH = combo.shape[0]
    a = combo.reshape(KH // P, P, NPS)                   # (kt, p, n)
    return np.ascontiguousarray(a.transpose(1, 0, 2))    # (p, kt, n)


def host_prep(x, qweight, qzeros, scales, g_idx, bias, m_split, n_split):
    """Dequantize W, build fp16 Strassen A/B combos, pack per core."""
    M_full, K = x.shape
    G, N_full = scales.shape
    Mc = M_full // m_split
    Nc = N_full // n_split
    KH, MHc, NHc = K // 2, Mc // 2, Nc // 2

    shifts = (np.arange(JP, dtype=np.int32) * 4)
    w = ((qweight[:, None, :] >> shifts[None, :, None]) & 15).reshape(K, N_full)
    z = ((qzeros[:, :, None] >> shifts[None, None, :]) & 15).reshape(G, N_full) + 1
    cg = np.asarray(g_idx[::GROUP])
    assert np.array_equal(np.repeat(cg, GROUP), np.asarray(g_idx)), \
        "g_idx must be uniform within 128-wide k chunks"
    iw = (w - z[cg].repeat(GROUP, axis=0)).astype(np.float32)
    W16 = (iw * np.asarray(scales, np.float32)[cg].repeat(GROUP, axis=0)
           ).astype(np.float16)

    x = np.asarray(x)
    bias = np.asarray(bias)

    a_shards = []
    for mi in range(m_split):
        xm = x[mi * Mc:(mi + 1) * Mc]
        A11 = xm[:MHc, :KH]; A12 = xm[:MHc, KH:]
        A21 = xm[MHc:, :KH]; A22 = xm[MHc:, KH:]
        combos = (A11 + A22, A21 + A22, A11, A22,
                  A11 + A12, A21 - A11, A12 - A22)
        arr = np.empty((NPROD, MHc // P, P, KH // P, P), np.float16)
        for i, c in enumerate(combos):
            arr[i] = _pack_a(np.ascontiguousarray(c))
        a_shards.append(arr)

    b_shards = []
    for ni in range(n_split):
        Wc = W16[:, ni * Nc:(ni + 1) * Nc]
        B11 = Wc[:KH, :NHc]; B12 = Wc[:KH, NHc:]
        B21 = Wc[KH:, :NHc]; B22 = Wc[KH:, NHc:]
        combos = (B11 + B22, B11, B12 - B22, B21 - B11,
                  B22, B11 + B12, B21 + B22)
        arr = np.empty((NPROD, P, KH // P, NHc), np.float16)
        for i, c in enumerate(combos):
            arr[i] = _pack_b(np.ascontiguousarray(c))
        b_shards.append(arr)

    in_maps = []
    for mi in range(m_split):
        for ni in range(n_split):
            in_maps.append({
                "ac": a_shards[mi],
                "bc": b_shards[ni],
                "bs": np.ascontiguousarray(
                    np.broadcast_to(bias[ni * Nc:(ni + 1) * Nc], (P, Nc))
                ),
            })
    return in_maps, Mc, Nc


_PROGRAM_CACHE = {}


def _get_program(K, M, N):
    key = (K, M, N)
    if key not in _PROGRAM_CACHE:
        _PROGRAM_CACHE[key] = build_program(K, M, N)
    return _PROGRAM_CACHE[key]


def kernel(x, qweight, qzeros, scales, g_idx, bias, trace=False, trace_kwargs=None):
    m_split, n_split = 2, 4
    x = np.asarray(x)
    qweight = np.asarray(qweight)
    qzeros = np.asarray(qzeros)
    scales = np.asarray(scales)
    g_idx = np.asarray(g_idx)
    bias = np.asarray(bias)
    M_full, K = x.shape
    N_full = scales.shape[1]
    in_maps, M, N = host_prep(x, qweight, qzeros, scales, g_idx, bias,
                              m_split, n_split)
    nc = _get_program(K, M, N)
    kw = {}
    if trace:
        kw = dict(trace=True, **(trace_kwargs or {}))
    rb = run_bass_kernel_spmd(nc, in_maps, list(range(m_split * n_split)), **kw)
    out = np.empty((M_full, N_full), dtype=np.float16)
    ci = 0
    for mi in range(m_split):
        for ni in range(n_split):
            out[mi * M:(mi + 1) * M, ni * N:(ni + 1) * N] = rb.results[ci]["out"]
            ci += 1
    kernel.last_results = rb
    return out
